# revision 12
# baseline (speedup 1.0000x reference)
"""BiDAF block kernel for Trainium2 (Bass/Tile), 8 cores = 4 batch-pairs x 2 LSTM
directions.

Sharding: batch 32 -> 4 groups of 8; each group owns a core PAIR (even=forward,
odd=backward). Backward cores receive the context time-REVERSED on the host, so
the same SPMD program computes both directions (attention is seq-permutation
equivariant; the scan always runs "forward" over its local time order).

Between layers the pair exchanges hidden states with a masked 2-rank
ReduceScatter (each core contributes its h time-reversed into the shard its
peer keeps; its own shard contribution is zeroed by a per-core mask input).

The LSTM recurrent matmul runs in fp8-e4m3 DoubleRow mode (2x PE throughput,
K=256/instr), with whh scaled x64 to avoid fp8 subnormals; xg is prescaled x64
(folded into wih on the host) and the gate activations divide by 64 via the
activation scale. Cell state + elementwise are bf16 (validated ~8e-4 rel err).

Gate columns are host-permuted to [o | f | i | g~] so sigmoids merge and the
tail pipeline starts early.
"""

from contextlib import ExitStack

import numpy as np

import concourse.bacc as bacc
import concourse.bass as bass
import concourse.mybir as mybir
import concourse.tile as tile
from concourse.bass import ds, ts
from concourse.masks import make_identity

F32 = mybir.dt.float32
BF16 = mybir.dt.bfloat16
FP8 = mybir.dt.float8e4
AF = mybir.ActivationFunctionType
ALU = mybir.AluOpType
AX = mybir.AxisListType
PM = mybir.MatmulPerfMode
P = 128

B_FULL, T, QLEN, H = 32, 384, 64, 768
H2, H4 = 2 * H, 4 * H
KH = H // P            # 6
KH2 = H2 // P          # 12
KH4 = H4 // P          # 24
KPAIR = KH // 2        # 3 fp8 k-tile pairs
N_CORES = 8
B = 8                  # local batch (one group)
PB = 16                # padded batch for fp8 DoubleRow lhsT (free%16==0)
SC = 64                # steps per hw-loop iteration
NIT = T // SC
SCALE = 64.0           # whh/xg prescale to keep fp8 out of subnormals
ISC = 1.0 / SCALE

# gate column blocks after host permutation [f i o g~] (torch order is i,f,g,o)
F0, I0, O0, G0 = 0, H, 2 * H, 3 * H

LAYERS = ("l1", "l2", "lo")


def _emit_pack(nc, tc, prev, hT8, hacc, hrev, j):
    hTa, hTb = prev
    AFc = mybir.ActivationFunctionType.Copy
    nc.scalar.activation(hT8[:, 0, :, :B], hTa, AFc)
    nc.scalar.activation(hT8[:, 1:3, :, :B].rearrange("p a b c -> p (a b) c"), hTb, AFc)
    nc.vector.tensor_copy(hacc[:, 0:2, :, j], hTa)
    nc.vector.tensor_copy(hacc[:, 2:6, :, j], hTb)
    nc.vector.tensor_copy(hrev[:, 0:2, :, SC - 1 - j], hTa)
    nc.vector.tensor_copy(hrev[:, 2:6, :, SC - 1 - j], hTb)


def build(ctx, tc, io, cfg):
    nc = tc.nc
    b_att = cfg["b_att"]
    p_bias = cfg["p_bias"]

    # ---------------- DRAM scratch ----------------
    dram = ctx.enter_context(tc.tile_pool(name="dram", bufs=1, space="DRAM"))
    gT_d = dram.tile([B, KH4, P, T], BF16)
    xg_d = {l: dram.tile([T, B, H4], FP8, name=f"xg_{l}") for l in LAYERS}
    mT_d = {l: dram.tile([P, KH, B, T], BF16, name=f"mT_{l}") for l in LAYERS}
    rsin_d = {l: dram.tile([2, P, KH, B, T], BF16, name=f"rsin_{l}") for l in LAYERS}
    rsout_d = {l: dram.tile([P, KH, B, T], BF16, name=f"rsout_{l}") for l in LAYERS}

    # ---------------- constants ----------------
    cpool = ctx.enter_context(tc.tile_pool(name="const", bufs=1))
    ident = cpool.tile([P, P], F32)
    make_identity(nc, ident)
    ident_bf = cpool.tile([P, P], BF16)
    make_identity(nc, ident_bf)
    w_cq_sb = cpool.tile([P, KH], F32)
    nc.sync.dma_start(w_cq_sb, io["w_cq_"])
    w_c_sb = cpool.tile([P, KH], F32)
    nc.sync.dma_start(w_c_sb, io["w_c_"])
    w_q_sb = cpool.tile([P, KH], F32)
    nc.sync.dma_start(w_q_sb, io["w_q_"])
    ones_sb = cpool.tile([P, 1], F32)
    nc.vector.memset(ones_sb, 1.0)
    ones_row = cpool.tile([1, P], F32)
    nc.vector.memset(ones_row, 1.0)
    mask_sb = cpool.tile([P, 2], F32)
    nc.sync.dma_start(mask_sb, io["mask"])
    onehot_sb = cpool.tile([P, 2, PB], FP8)
    nc.sync.dma_start(onehot_sb, io["onehot"])
    pw_sb = {}
    for nm, kc in (("pwg", KH4), ("pwm2", KH2), ("pwmo", KH2)):
        pw_sb[nm] = cpool.tile([P, kc], BF16, name=f"pw_{nm}")
        nc.sync.dma_start(pw_sb[nm], io[nm])

    # ================ Phase 1: attention -> gT_d ================
    with tc.tile_pool(name="att", bufs=2) as att, \
         tc.tile_pool(name="att_ps", bufs=4, space="PSUM") as aps:
        TC = T // P
        for b in range(B):
            cT_sb = att.tile([P, KH, T], F32, tag="cT")
            nc.sync.dma_start(cT_sb, io["cT"][b].rearrange("kc p t -> p kc t"))
            cna_sb = att.tile([P, TC, H], F32, tag="cna")
            nc.sync.dma_start(cna_sb, io["c"][b].rearrange("(io p) h -> p io h", p=P))
            q_sb = att.tile([QLEN, H], F32, tag="q")
            nc.sync.dma_start(q_sb, io["q"][b])
            qT_sb = att.tile([P, KH, QLEN], F32, tag="qT")
            nc.sync.dma_start(qT_sb, io["qT"][b].rearrange("kc p t -> p kc t"))

            cw_sb = att.tile([P, KH, T], F32, tag="cw")
            for k in range(KH):
                nc.vector.tensor_tensor(
                    cw_sb[:, k], cT_sb[:, k],
                    w_cq_sb[:, k, None].to_broadcast((P, T)), ALU.mult)

            sq_ps = aps.tile([QLEN, 1], F32, tag="aps")
            for k in range(KH):
                nc.tensor.matmul(sq_ps, lhsT=qT_sb[:, k], rhs=w_q_sb[:, k, None],
                                 start=(k == 0), stop=(k == KH - 1))
            sq_col = att.tile([QLEN, 1], F32, tag="sq_col")
            nc.scalar.activation(sq_col, sq_ps, AF.Copy, bias=float(b_att))
            sqT_ps = aps.tile([1, QLEN], F32, tag="aps")
            nc.tensor.transpose(sqT_ps, sq_col, ident[:QLEN, :QLEN])
            sq_row = att.tile([1, QLEN], F32, tag="sq_row")
            nc.scalar.activation(sq_row, sqT_ps, AF.Copy)

            a_sb = att.tile([P, TC, QLEN], F32, tag="a")
            e2_sb = att.tile([P, TC], F32, tag="e2")
            for ic in range(TC):
                s_ps = aps.tile([P, QLEN], F32, tag="aps")
                for k in range(KH):
                    nc.tensor.matmul(s_ps, lhsT=cw_sb[:, k, ts(ic, P)],
                                     rhs=qT_sb[:, k],
                                     start=(k == 0), stop=False)
                nc.tensor.matmul(s_ps, lhsT=ones_row, rhs=sq_row,
                                 start=False, stop=True)
                sc_ps = aps.tile([P, 1], F32, tag="aps")
                for k in range(KH):
                    nc.tensor.matmul(sc_ps, lhsT=cT_sb[:, k, ts(ic, P)],
                                     rhs=w_c_sb[:, k, None],
                                     start=(k == 0), stop=(k == KH - 1))
                sc_sb = att.tile([P, 1], F32, tag="sc_sb")
                nc.scalar.activation(sc_sb, sc_ps, AF.Copy)
                s_sb = att.tile([P, QLEN], F32, tag="s_sb")
                nc.vector.tensor_tensor(s_sb, s_ps, sc_sb.to_broadcast((P, QLEN)), ALU.add)

                nmx = att.tile([P, 1], F32, tag="nmx")
                nc.vector.reduce_max(nmx, s_sb, axis=AX.X, negate=True)
                nc.scalar.activation(a_sb[:, ic], s_sb, AF.Exp, bias=nmx)
                ssum = att.tile([P, 1], F32, tag="ssum")
                nc.vector.reduce_sum(ssum, a_sb[:, ic], axis=AX.X)
                rs = att.tile([P, 1], F32, tag="rs")
                nc.vector.reciprocal(rs, ssum)
                nc.vector.tensor_scalar_mul(a_sb[:, ic], a_sb[:, ic], rs)

                mx = att.tile([P, 1], F32, tag="mx")
                nc.vector.reduce_max(mx, s_sb, axis=AX.X)
                nc.scalar.activation(e2_sb[:, ic, None], mx, AF.Exp)

            bsum_ps = aps.tile([1, TC], F32, tag="aps")
            nc.tensor.matmul(bsum_ps, lhsT=ones_sb, rhs=e2_sb, start=True, stop=True)
            tot = att.tile([1, 1], F32, tag="tot")
            nc.vector.reduce_sum(tot, bsum_ps, axis=AX.X)
            totb_ps = aps.tile([P, 1], F32, tag="aps")
            nc.tensor.matmul(totb_ps, lhsT=ones_row, rhs=tot, start=True, stop=True)
            rtot = att.tile([P, 1], F32, tag="rtot")
            nc.vector.reciprocal(rtot, totb_ps)
            bw_sb = att.tile([P, TC], F32, tag="bw")
            nc.vector.tensor_scalar_mul(bw_sb, e2_sb, rtot)

            q2c_sb = att.tile([1, H], F32, tag="q2c_sb")
            for half in range(2):
                q2c_ps = aps.tile([1, H // 2], F32, tag="aps")
                for ic in range(TC):
                    nc.tensor.matmul(q2c_ps, lhsT=bw_sb[:, ic, None],
                                     rhs=cna_sb[:, ic, ds(half * (H // 2), H // 2)],
                                     start=(ic == 0), stop=(ic == TC - 1))
                nc.scalar.activation(q2c_sb[:, ds(half * (H // 2), H // 2)], q2c_ps, AF.Copy)
            q2cT_sb = att.tile([P, KH], F32, tag="q2cT")
            for k in range(KH):
                q2cT_ps = aps.tile([P, 1], F32, tag="aps")
                nc.tensor.transpose(q2cT_ps, q2c_sb[:, ts(k, P)], ident[:1, :1])
                nc.scalar.activation(q2cT_sb[:, k, None], q2cT_ps, AF.Copy)

            aT_sb = att.tile([QLEN, TC, P], F32, tag="aT")
            for ic in range(TC):
                aT_ps = aps.tile([QLEN, P], F32, tag="aps")
                nc.tensor.transpose(aT_ps, a_sb[:, ic], ident)
                nc.scalar.activation(aT_sb[:, ic], aT_ps, AF.Copy)

            aT_flat = aT_sb.rearrange("q a b -> q (a b)")
            for fc in range(KH):
                c2q_ps = aps.tile([P, T], F32, tag="aps")
                nc.tensor.matmul(c2q_ps, lhsT=q_sb[:, ts(fc, P)], rhs=aT_flat,
                                 start=True, stop=True)
                c2q_sb = att.tile([P, T], F32, tag="c2q_sb")
                nc.scalar.activation(c2q_sb, c2q_ps, AF.Copy)
                c2qb_sb = att.tile([P, T], BF16, tag="c2qb_sb")
                nc.scalar.activation(c2qb_sb, c2q_ps, AF.Copy)
                g3_sb = att.tile([P, T], BF16, tag="g3")
                nc.vector.tensor_tensor(g3_sb, cT_sb[:, fc], c2q_sb, ALU.mult)
                g4_sb = att.tile([P, T], BF16, tag="g4")
                nc.vector.tensor_tensor(
                    g4_sb, cT_sb[:, fc],
                    q2cT_sb[:, fc, None].to_broadcast((P, T)), ALU.mult)
                nc.sync.dma_start(gT_d[b, fc], io["cT_bf"][b, fc])
                nc.sync.dma_start(gT_d[b, KH + fc], c2qb_sb)
                nc.sync.dma_start(gT_d[b, 2 * KH + fc], g3_sb)
                nc.sync.dma_start(gT_d[b, 3 * KH + fc], g4_sb)

    # ================ Phase 2: three layers ================
    for li, lname in enumerate(LAYERS):
        KC = KH4 if li == 0 else KH2

        # ---- 2a: xg = src @ wihT(x64, col-permuted) -> xg_d[lname] ----
        with tc.tile_pool(name=f"prj{li}", bufs=2) as prj, \
             tc.tile_pool(name=f"prjw{li}", bufs=1) as prjw, \
             tc.tile_pool(name=f"prj{li}_ps", bufs=2, space="PSUM") as pps:
            halves = 2 if li == 0 else 1
            HN = H4 // halves
            NB = HN // 512
            for half in range(halves):
                w_sb = prjw.tile([P, KC, HN], BF16, tag="wih")
                nc.sync.dma_start(
                    w_sb,
                    io[f"{lname}_wihT"][:, :, ds(half * HN, HN)]
                    .rearrange("kc p n -> p kc n"))
                for b in range(B):
                    for mc in range(T // P):
                        inp_sb = prj.tile([P, KC, P], BF16, tag="inp")
                        if li == 0:
                            nc.sync.dma_start(
                                inp_sb,
                                gT_d[b, :, :, ts(mc, P)].rearrange("kc p t -> p kc t"))
                        else:
                            prev = LAYERS[li - 1]
                            nc.sync.dma_start(inp_sb[:, :KH], mT_d[prev][:, :, b, ts(mc, P)])
                            nc.gpsimd.dma_start(inp_sb[:, KH:], rsout_d[prev][:, :, b, ts(mc, P)])
                        for n in range(NB):
                            xg_ps = pps.tile([P, 512], F32, tag="xg")
                            for k in range(KC):
                                nc.tensor.matmul(
                                    xg_ps, lhsT=inp_sb[:, k],
                                    rhs=w_sb[:, k, ts(n, 512)],
                                    start=(k == 0), stop=(k == KC - 1))
                            xg_sb = prj.tile([P, 512], FP8, tag="xg_sb")
                            nc.scalar.activation(xg_sb, xg_ps, AF.Copy)
                            off = half * HN + n * 512
                            nc.sync.dma_start(
                                xg_d[lname][ts(mc, P), b, ds(off, 512)],
                                xg_sb)

        # ---- 2b: scan (always "forward" in local time) ----
        with tc.tile_pool(name=f"whh{li}", bufs=1) as whhp, \
             tc.tile_pool(name=f"st{li}", bufs=1) as stp, \
             tc.tile_pool(name=f"scan{li}", bufs=2) as scp, \
             tc.tile_pool(name=f"xg{li}", bufs=SC, space="SBUF") as xgp, \
             tc.tile_pool(name=f"scan{li}_ps", bufs=1, space="PSUM") as sps, \
             tc.tile_pool(name=f"tp{li}_ps", bufs=1, space="PSUM") as tps:
            whh_sb = whhp.tile([P, KPAIR, 2, H4], FP8, name="whh_sb")
            nc.sync.dma_start(whh_sb, io[f"{lname}_whh8"].rearrange("a b p n -> p a b n"))

            hT8 = stp.tile([P, KPAIR, 2, PB], FP8, name="hT8")
            nc.vector.memset(hT8, 0.0)
            c_st = stp.tile([B, H], BF16, name="c_st")
            nc.vector.memset(c_st, 0.0)
            xring = []
            for j in range(8):
                xt = stp.tile([P, 2, H4], FP8, name=f"xring{j}")
                nc.vector.memset(xt, 0.0)
                xring.append(xt)

            with tc.For_i(0, NIT, 1) as iv:
                def issue_xg(j0):
                    for j in range(j0, min(j0 + 8, SC)):
                        (nc.sync if j % 2 == 0 else nc.gpsimd).dma_start(
                            xring[j % 8][:B, 0],
                            xg_d[lname][ds(iv * SC + j, 1)].rearrange("a b n -> (a b) n"))
                issue_xg(0)

                hacc = scp.tile([P, KH, B, SC], BF16, tag="hacc", name="hacc")
                hrev = scp.tile([P, KH, B, SC], BF16, tag="hrev", name="hrev")

                prev = None  # (hTa, hTb) transposes of previous step pending pack
                for j in range(SC):
                    gA = sps.tile([PB, 3, 512], F32, tag="gA", name="gA")
                    gB = sps.tile([PB, 3, 512], F32, tag="gB", name="gB")
                    xt = xring[j % 8]
                    # fold xg into PSUM via one-hot lhsT (independent of h -> fills tail stall)
                    for nb in range(3):
                        nc.tensor.matmul(gA[:, nb], lhsT=onehot_sb, rhs=xt[:, :, ts(nb, 512)],
                                         start=True, stop=False, perf_mode=PM.DoubleRow)
                    for nb in range(3):
                        nc.tensor.matmul(gB[:, nb], lhsT=onehot_sb, rhs=xt[:, :, ts(3 + nb, 512)],
                                         start=True, stop=False, perf_mode=PM.DoubleRow)
                    # previous step's transposes + packs (emitted here so this step's
                    # folds precede them on PE; kp mms below wait on these packs)
                    if prev is not None:
                        _emit_pack(nc, tc, prev, hT8, hacc, hrev, j - 1)
                        prev = None
                    # recurrent accumulation
                    for kp in range(KPAIR):
                        for g, nbl in ((gA, 0), (gB, 3)):
                            for nb in range(3):
                                nc.tensor.matmul(g[:, nb], lhsT=hT8[:, kp],
                                                 rhs=whh_sb[:, kp, :, ts(nbl + nb, 512)],
                                                 start=False, stop=(kp == KPAIR - 1),
                                                 perf_mode=PM.DoubleRow)
                    if j + 8 < SC:
                        ((nc.sync if j % 2 == 0 else nc.gpsimd)).dma_start(
                            xring[j % 8][:B, 0],
                            xg_d[lname][ds(iv * SC + j + 8, 1)].rearrange("a b n -> (a b) n"))
                    gAf = gA[:B].rearrange("p a n -> p (a n)")
                    gBf = gB[:B].rearrange("p a n -> p (a n)")
                    fi_bf = scp.tile([B, 2, H], BF16, tag="fi", name="fi_bf")
                    nc.scalar.activation(fi_bf.rearrange("b a h -> b (a h)"), gAf,
                                         AF.Sigmoid, scale=ISC)
                    o_bf = scp.tile([B, H], BF16, tag="o_bf", name="o_bf")
                    # tail: Act order sf,si,g0,g1,so,tc0,tc1 ; DVE cf/ig/ca/h per sub
                    gs_t, ig_t, tc_t, hs_t = [], [], [], []
                    for lo_c, wd, si in ((0, 256, 0), (256, 512, 1)):
                        cs = c_st[:, ds(lo_c, wd)]
                        nc.vector.tensor_tensor(cs, fi_bf[:, 0, ds(lo_c, wd)], cs, ALU.mult)
                        gs = scp.tile([B, wd], BF16, tag=f"gs{si}", name=f"gs{si}")
                        nc.scalar.activation(gs, gBf[:, ds(H + lo_c, wd)], AF.Tanh, scale=ISC)
                        gs_t.append(gs)
                    for lo_c, wd, si in ((0, 256, 0), (256, 512, 1)):
                        cs = c_st[:, ds(lo_c, wd)]
                        ig = scp.tile([B, wd], BF16, tag=f"ig{si}", name=f"ig{si}")
                        nc.vector.tensor_tensor(ig, fi_bf[:, 1, ds(lo_c, wd)], gs_t[si], ALU.mult)
                        nc.vector.tensor_tensor(cs, cs, ig, ALU.add)
                    nc.scalar.activation(o_bf, gBf[:, :H], AF.Sigmoid, scale=ISC)
                    subs = []
                    for lo_c, wd, si in ((0, 256, 0), (256, 512, 1)):
                        cs = c_st[:, ds(lo_c, wd)]
                        tcs = scp.tile([B, wd], BF16, tag=f"tc{si}", name=f"tc{si}")
                        nc.scalar.activation(tcs, cs, AF.Tanh)
                        hs = scp.tile([B, wd], BF16, tag=f"hs{si}", name=f"hs{si}")
                        nc.vector.tensor_tensor(hs, o_bf[:, ds(lo_c, wd)], tcs, ALU.mult)
                        subs.append(hs)
                    hTa = tps.tile([P, 2, B], BF16, tag="hTa", name="hTa")
                    hTb = tps.tile([P, 4, B], BF16, tag="hTb", name="hTb")
                    for k in (0, 1):
                        nc.tensor.transpose(hTa[:, k], subs[0][:, ts(k, P)], ident_bf[:B, :B])
                    for k in range(4):
                        nc.tensor.transpose(hTb[:, k], subs[1][:, ts(k, P)], ident_bf[:B, :B])
                    prev = (hTa, hTb)
                _emit_pack(nc, tc, prev, hT8, hacc, hrev, SC - 1)

                # flush: own order -> mT_d ; reversed+masked -> rsin_d shards
                nc.scalar.dma_start(mT_d[lname][:, :, :, ds(iv * SC, SC)], hacc)
                hs0 = scp.tile([P, KH, B, SC], BF16, tag="hs0", name="hs0")
                nc.vector.tensor_scalar_mul(hs0, hrev, mask_sb[:, 0, None])
                hs1 = scp.tile([P, KH, B, SC], BF16, tag="hs1", name="hs1")
                nc.vector.tensor_scalar_mul(hs1, hrev, mask_sb[:, 1, None])
                nc.scalar.dma_start(rsin_d[lname][0][:, :, :, ds(T - SC - iv * SC, SC)], hs0)
                nc.scalar.dma_start(rsin_d[lname][1][:, :, :, ds(T - SC - iv * SC, SC)], hs1)

        # ---- 2c: pair exchange ----
        nc.gpsimd.collective_compute(
            "ReduceScatter", mybir.AluOpType.add,
            replica_groups=[[0, 1], [2, 3], [4, 5], [6, 7]],
            ins=[rsin_d[lname].rearrange("s p k b t -> (s p) (k b t)").opt()],
            outs=[rsout_d[lname].rearrange("p k b t -> p (k b t)").opt()],
        )

    # ================ Phase 3: p readout ================
    with tc.tile_pool(name="out", bufs=3) as osb, \
         tc.tile_pool(name="out_ps", bufs=2, space="PSUM") as ops:
        for b in range(B):
            p_ps = ops.tile([1, T], F32, tag="p_ps", name="p_ps")
            for k in range(KH4):
                gt = osb.tile([P, T], BF16, tag="gt")
                nc.sync.dma_start(gt, gT_d[b, k])
                nc.tensor.matmul(p_ps, lhsT=pw_sb["pwg"][:, k, None],
                                 rhs=gt, start=(k == 0), stop=False)
            for nm, own, peer in (("pwm2", mT_d["l2"], rsout_d["l2"]),
                                  ("pwmo", mT_d["lo"], rsout_d["lo"])):
                for k in range(KH2):
                    mt = osb.tile([P, T], BF16, tag=f"mt_{nm}")
                    src = own if k < KH else peer
                    nc.sync.dma_start(mt, src[:, k % KH, b])
                    nc.tensor.matmul(p_ps, lhsT=pw_sb[nm][:, k, None],
                                     rhs=mt, start=False,
                                     stop=(nm == "pwmo" and k == KH2 - 1))
            p_sb = osb.tile([1, T], F32, tag="p_sb")
            nc.scalar.activation(p_sb, p_ps, AF.Copy, bias=float(p_bias))
            nc.sync.dma_start(io["p"][b], p_sb)


# ==================== host-side driver ====================

_GATE_PERM = None


def _gate_perm():
    """column permutation: new [o f i g~] from torch (i,f,g,o)."""
    global _GATE_PERM
    if _GATE_PERM is None:
        f = np.arange(H, 2 * H)
        i = np.arange(0, H)
        o = np.arange(3 * H, 4 * H)
        g = np.arange(2 * H, 3 * H)
        _GATE_PERM = np.concatenate([f, i, o, g])
    return _GATE_PERM


def _prep_core(inputs, core):
    import ml_dtypes
    bf16 = ml_dtypes.bfloat16
    f8 = ml_dtypes.float8_e4m3
    f32 = np.float32
    pair, is_b = core // 2, core % 2
    lo, hi = pair * B, (pair + 1) * B
    d = "b" if is_b else "f"
    perm = _gate_perm()

    m = {}
    cs = np.asarray(inputs["c"][lo:hi], f32)
    if is_b:
        cs = cs[:, ::-1]
    qs = np.asarray(inputs["q"][lo:hi], f32)
    cT = np.ascontiguousarray(cs.transpose(0, 2, 1).reshape(B, KH, P, T))
    m["c"] = np.ascontiguousarray(cs)
    m["q"] = np.ascontiguousarray(qs)
    m["cT"] = cT
    m["cT_bf"] = cT.astype(bf16)
    m["qT"] = np.ascontiguousarray(qs.transpose(0, 2, 1).reshape(B, KH, P, QLEN))

    m["w_cq_"] = np.ascontiguousarray(inputs["w_att_cq"].reshape(KH, P).T).astype(f32)
    m["w_c_"] = np.ascontiguousarray(inputs["w_att_c"].reshape(KH, P).T).astype(f32)
    m["w_q_"] = np.ascontiguousarray(inputs["w_att_q"].reshape(KH, P).T).astype(f32)

    for lname in LAYERS:
        wih = np.asarray(inputs[f"{lname}{d}_wih"], f32)   # [4H, in]
        whh = np.asarray(inputs[f"{lname}{d}_whh"], f32)   # [4H, H]
        ind = wih.shape[1]
        wihT = wih.T[:, perm] * SCALE                      # [in, 4H] x64, col-perm
        if lname != "l1":
            # rows: own-dir half first, peer half second
            top, bot = wihT[:H], wihT[H:]
            wihT = np.concatenate([bot, top], 0) if is_b else wihT
        m[f"{lname}_wihT"] = np.ascontiguousarray(
            wihT.reshape(ind // P, P, H4)).astype(bf16)
        whhT = whh.T[:, perm] * SCALE                      # [H, 4H]
        m[f"{lname}_whh8"] = np.ascontiguousarray(
            whhT.reshape(KPAIR, 2, P, H4)).astype(f8)

    if is_b:
        wg, wm = np.asarray(inputs["p2_wg"], f32), np.asarray(inputs["p2_wm"], f32)
        wm_loc = np.concatenate([wm[H:], wm[:H]])
        wm2, wmo = np.zeros(H2, f32), wm_loc
    else:
        wg, wm = np.asarray(inputs["p1_wg"], f32), np.asarray(inputs["p1_wm"], f32)
        wm2, wmo = wm, np.zeros(H2, f32)
    m["pwg"] = np.ascontiguousarray(wg.reshape(KH4, P).T).astype(bf16)
    m["pwm2"] = np.ascontiguousarray(wm2.reshape(KH2, P).T).astype(bf16)
    m["pwmo"] = np.ascontiguousarray(wmo.reshape(KH2, P).T).astype(bf16)

    mk = np.zeros((P, 2), f32)
    mk[:, 1 - is_b] = 1.0   # even core contributes shard1; odd shard0
    m["mask"] = mk
    oh = np.zeros((P, 2, PB), np.float32)
    for k in range(B):
        oh[k, 0, k] = 1.0
    m["onehot"] = oh.astype(f8)
    return m


def declare_io(nc):
    io = {}

    def inp(name, shape, dt=F32):
        io[name] = nc.declare_dram_parameter(name, list(shape), dt, isOutput=False).ap()

    inp("c", (B, T, H))
    inp("q", (B, QLEN, H))
    inp("cT", (B, KH, P, T))
    inp("cT_bf", (B, KH, P, T), BF16)
    inp("qT", (B, KH, P, QLEN))
    inp("w_cq_", (P, KH))
    inp("w_c_", (P, KH))
    inp("w_q_", (P, KH))
    inp("mask", (P, 2))
    inp("onehot", (P, 2, PB), FP8)
    for lname in LAYERS:
        ind = H4 if lname == "l1" else H2
        inp(f"{lname}_wihT", (ind // P, P, H4), BF16)
        inp(f"{lname}_whh8", (KPAIR, 2, P, H4), FP8)
    inp("pwg", (P, KH4), BF16)
    inp("pwm2", (P, KH2), BF16)
    inp("pwmo", (P, KH2), BF16)
    io["p"] = nc.declare_dram_parameter("p", [B, T], F32, isOutput=True).ap()
    return io


def kernel(**inputs):
    from concourse.bass_utils import run_bass_kernel_spmd

    cfg = {
        "b_att": float(inputs["b_att_c"]) + float(inputs["b_att_q"]) + float(inputs["b_att_cq"]),
        "p_bias": 0.0,  # per-core below
    }
    p1_b = float(inputs["p1_bg"]) + float(inputs["p1_bm"])
    p2_b = float(inputs["p2_bg"]) + float(inputs["p2_bm"])

    nc = bacc.Bacc("TRN2", target_bir_lowering=False, debug=False, num_devices=N_CORES)
    io = declare_io(nc)
    cfg["p_bias"] = 0.0
    with tile.TileContext(nc) as tc, ExitStack() as ctx:
        build(ctx, tc, io, cfg)
    nc.compile()

    in_maps = [_prep_core(inputs, core) for core in range(N_CORES)]

    import os as _os
    import time as _time

    def _run():
        try:
            return run_bass_kernel_spmd(nc, in_maps, core_ids=list(range(N_CORES)))
        except Exception:
            if _os.environ.get("BASS_TRACE"):
                _os.environ["BASS_NEVER_TRACE"] = "1"
                return run_bass_kernel_spmd(nc, in_maps, core_ids=list(range(N_CORES)))
            raise

    t0 = _time.time()
    res = _run()
    globals()["LAST_RUN"] = res
    globals()["LAST_EXEC_WALL"] = _time.time() - t0
    t0 = _time.time()
    res2 = _run()
    globals()["WARM_EXEC_WALL"] = _time.time() - t0
    if res2.exec_time_ns is not None:
        globals()["LAST_RUN"] = res2
    res = res2

    p1 = np.concatenate([res.results[2 * g]["p"] + p1_b for g in range(4)], axis=0)
    p2 = np.concatenate([res.results[2 * g + 1]["p"][:, ::-1] + p2_b for g in range(4)], axis=0)
    return p1, p2


# revision 14
# speedup vs baseline: 1.0166x; 1.0166x over previous
"""BiDAF block kernel for Trainium2 (Bass/Tile), 8 cores = 4 batch-pairs x 2 LSTM
directions.

Sharding: batch 32 -> 4 groups of 8; each group owns a core PAIR (even=forward,
odd=backward). Backward cores receive the context time-REVERSED on the host, so
the same SPMD program computes both directions (attention is seq-permutation
equivariant; the scan always runs "forward" over its local time order).

Between layers the pair exchanges hidden states with a masked 2-rank
ReduceScatter (each core contributes its h time-reversed into the shard its
peer keeps; its own shard contribution is zeroed by a per-core mask input).

The LSTM recurrent matmul runs in fp8-e4m3 DoubleRow mode (2x PE throughput,
K=256/instr), with whh scaled x64 to avoid fp8 subnormals; xg is prescaled x64
(folded into wih on the host) and the gate activations divide by 64 via the
activation scale. Cell state + elementwise are bf16 (validated ~8e-4 rel err).

Gate columns are host-permuted to [o | f | i | g~] so sigmoids merge and the
tail pipeline starts early.
"""

from contextlib import ExitStack

import numpy as np

import concourse.bacc as bacc
import concourse.bass as bass
import concourse.mybir as mybir
import concourse.tile as tile
from concourse.bass import ds, ts
from concourse.masks import make_identity

F32 = mybir.dt.float32
BF16 = mybir.dt.bfloat16
FP8 = mybir.dt.float8e4
AF = mybir.ActivationFunctionType
ALU = mybir.AluOpType
AX = mybir.AxisListType
PM = mybir.MatmulPerfMode
P = 128

B_FULL, T, QLEN, H = 32, 384, 64, 768
H2, H4 = 2 * H, 4 * H
KH = H // P            # 6
KH2 = H2 // P          # 12
KH4 = H4 // P          # 24
KPAIR = KH // 2        # 3 fp8 k-tile pairs
N_CORES = 8
B = 8                  # local batch (one group)
PB = 16                # padded batch for fp8 DoubleRow lhsT (free%16==0)
SC = 64                # steps per hw-loop iteration
NIT = T // SC
SCALE = 64.0           # whh/xg prescale to keep fp8 out of subnormals
ISC = 1.0 / SCALE

# gate column blocks after host permutation [o f i g~] (torch order is i,f,g,o)
O0, F0, I0, G0 = 0, H, 2 * H, 3 * H

LAYERS = ("l1", "l2", "lo")


def _emit_pack(nc, tc, prev, hT8, hacc, hrev, j):
    hTa, hTb = prev
    AFc = mybir.ActivationFunctionType.Copy
    nc.scalar.activation(hT8[:, 0:2, :, :B].rearrange("p a b c -> p (a b) c"), hTa, AFc)
    nc.scalar.activation(hT8[:, 2, :, :B], hTb, AFc)
    nc.vector.tensor_copy(hacc[:, 0:4, :, j], hTa)
    nc.vector.tensor_copy(hacc[:, 4:6, :, j], hTb)
    nc.vector.tensor_copy(hrev[:, 0:4, :, SC - 1 - j], hTa)
    nc.vector.tensor_copy(hrev[:, 4:6, :, SC - 1 - j], hTb)


def build(ctx, tc, io, cfg):
    nc = tc.nc
    b_att = cfg["b_att"]
    p_bias = cfg["p_bias"]

    # ---------------- DRAM scratch ----------------
    dram = ctx.enter_context(tc.tile_pool(name="dram", bufs=1, space="DRAM"))
    gT_d = dram.tile([B, KH4, P, T], BF16)
    xg_d = {l: dram.tile([T, B, H4], FP8, name=f"xg_{l}") for l in LAYERS}
    mT_d = {l: dram.tile([P, KH, B, T], BF16, name=f"mT_{l}") for l in LAYERS}
    rsin_d = {l: dram.tile([2, P, KH, B, T], BF16, name=f"rsin_{l}") for l in LAYERS}
    rsout_d = {l: dram.tile([P, KH, B, T], BF16, name=f"rsout_{l}") for l in LAYERS}

    # ---------------- constants ----------------
    cpool = ctx.enter_context(tc.tile_pool(name="const", bufs=1))
    ident = cpool.tile([P, P], F32)
    make_identity(nc, ident)
    ident_bf = cpool.tile([P, P], BF16)
    make_identity(nc, ident_bf)
    w_cq_sb = cpool.tile([P, KH], F32)
    nc.sync.dma_start(w_cq_sb, io["w_cq_"])
    w_c_sb = cpool.tile([P, KH], F32)
    nc.sync.dma_start(w_c_sb, io["w_c_"])
    w_q_sb = cpool.tile([P, KH], F32)
    nc.sync.dma_start(w_q_sb, io["w_q_"])
    ones_sb = cpool.tile([P, 1], F32)
    nc.vector.memset(ones_sb, 1.0)
    ones_row = cpool.tile([1, P], F32)
    nc.vector.memset(ones_row, 1.0)
    mask_sb = cpool.tile([P, 2], F32)
    nc.sync.dma_start(mask_sb, io["mask"])
    onehot_sb = cpool.tile([P, 2, PB], FP8)
    nc.sync.dma_start(onehot_sb, io["onehot"])
    pw_sb = {}
    for nm, kc in (("pwg", KH4), ("pwm2", KH2), ("pwmo", KH2)):
        pw_sb[nm] = cpool.tile([P, kc], BF16, name=f"pw_{nm}")
        nc.sync.dma_start(pw_sb[nm], io[nm])

    # ================ Phase 1: attention -> gT_d ================
    with tc.tile_pool(name="att", bufs=2) as att, \
         tc.tile_pool(name="att_ps", bufs=4, space="PSUM") as aps:
        TC = T // P
        for b in range(B):
            cT_sb = att.tile([P, KH, T], F32, tag="cT")
            nc.sync.dma_start(cT_sb, io["cT"][b].rearrange("kc p t -> p kc t"))
            cna_sb = att.tile([P, TC, H], F32, tag="cna")
            nc.sync.dma_start(cna_sb, io["c"][b].rearrange("(io p) h -> p io h", p=P))
            q_sb = att.tile([QLEN, H], F32, tag="q")
            nc.sync.dma_start(q_sb, io["q"][b])
            qT_sb = att.tile([P, KH, QLEN], F32, tag="qT")
            nc.sync.dma_start(qT_sb, io["qT"][b].rearrange("kc p t -> p kc t"))

            cw_sb = att.tile([P, KH, T], F32, tag="cw")
            for k in range(KH):
                nc.vector.tensor_tensor(
                    cw_sb[:, k], cT_sb[:, k],
                    w_cq_sb[:, k, None].to_broadcast((P, T)), ALU.mult)

            sq_ps = aps.tile([QLEN, 1], F32, tag="aps")
            for k in range(KH):
                nc.tensor.matmul(sq_ps, lhsT=qT_sb[:, k], rhs=w_q_sb[:, k, None],
                                 start=(k == 0), stop=(k == KH - 1))
            sq_col = att.tile([QLEN, 1], F32, tag="sq_col")
            nc.scalar.activation(sq_col, sq_ps, AF.Copy, bias=float(b_att))
            sqT_ps = aps.tile([1, QLEN], F32, tag="aps")
            nc.tensor.transpose(sqT_ps, sq_col, ident[:QLEN, :QLEN])
            sq_row = att.tile([1, QLEN], F32, tag="sq_row")
            nc.scalar.activation(sq_row, sqT_ps, AF.Copy)

            a_sb = att.tile([P, TC, QLEN], F32, tag="a")
            e2_sb = att.tile([P, TC], F32, tag="e2")
            for ic in range(TC):
                s_ps = aps.tile([P, QLEN], F32, tag="aps")
                for k in range(KH):
                    nc.tensor.matmul(s_ps, lhsT=cw_sb[:, k, ts(ic, P)],
                                     rhs=qT_sb[:, k],
                                     start=(k == 0), stop=False)
                nc.tensor.matmul(s_ps, lhsT=ones_row, rhs=sq_row,
                                 start=False, stop=True)
                sc_ps = aps.tile([P, 1], F32, tag="aps")
                for k in range(KH):
                    nc.tensor.matmul(sc_ps, lhsT=cT_sb[:, k, ts(ic, P)],
                                     rhs=w_c_sb[:, k, None],
                                     start=(k == 0), stop=(k == KH - 1))
                sc_sb = att.tile([P, 1], F32, tag="sc_sb")
                nc.scalar.activation(sc_sb, sc_ps, AF.Copy)
                s_sb = att.tile([P, QLEN], F32, tag="s_sb")
                nc.vector.tensor_tensor(s_sb, s_ps, sc_sb.to_broadcast((P, QLEN)), ALU.add)

                nmx = att.tile([P, 1], F32, tag="nmx")
                nc.vector.reduce_max(nmx, s_sb, axis=AX.X, negate=True)
                nc.scalar.activation(a_sb[:, ic], s_sb, AF.Exp, bias=nmx)
                ssum = att.tile([P, 1], F32, tag="ssum")
                nc.vector.reduce_sum(ssum, a_sb[:, ic], axis=AX.X)
                rs = att.tile([P, 1], F32, tag="rs")
                nc.vector.reciprocal(rs, ssum)
                nc.vector.tensor_scalar_mul(a_sb[:, ic], a_sb[:, ic], rs)

                mx = att.tile([P, 1], F32, tag="mx")
                nc.vector.reduce_max(mx, s_sb, axis=AX.X)
                nc.scalar.activation(e2_sb[:, ic, None], mx, AF.Exp)

            bsum_ps = aps.tile([1, TC], F32, tag="aps")
            nc.tensor.matmul(bsum_ps, lhsT=ones_sb, rhs=e2_sb, start=True, stop=True)
            tot = att.tile([1, 1], F32, tag="tot")
            nc.vector.reduce_sum(tot, bsum_ps, axis=AX.X)
            totb_ps = aps.tile([P, 1], F32, tag="aps")
            nc.tensor.matmul(totb_ps, lhsT=ones_row, rhs=tot, start=True, stop=True)
            rtot = att.tile([P, 1], F32, tag="rtot")
            nc.vector.reciprocal(rtot, totb_ps)
            bw_sb = att.tile([P, TC], F32, tag="bw")
            nc.vector.tensor_scalar_mul(bw_sb, e2_sb, rtot)

            q2c_sb = att.tile([1, H], F32, tag="q2c_sb")
            for half in range(2):
                q2c_ps = aps.tile([1, H // 2], F32, tag="aps")
                for ic in range(TC):
                    nc.tensor.matmul(q2c_ps, lhsT=bw_sb[:, ic, None],
                                     rhs=cna_sb[:, ic, ds(half * (H // 2), H // 2)],
                                     start=(ic == 0), stop=(ic == TC - 1))
                nc.scalar.activation(q2c_sb[:, ds(half * (H // 2), H // 2)], q2c_ps, AF.Copy)
            q2cT_sb = att.tile([P, KH], F32, tag="q2cT")
            for k in range(KH):
                q2cT_ps = aps.tile([P, 1], F32, tag="aps")
                nc.tensor.transpose(q2cT_ps, q2c_sb[:, ts(k, P)], ident[:1, :1])
                nc.scalar.activation(q2cT_sb[:, k, None], q2cT_ps, AF.Copy)

            aT_sb = att.tile([QLEN, TC, P], F32, tag="aT")
            for ic in range(TC):
                aT_ps = aps.tile([QLEN, P], F32, tag="aps")
                nc.tensor.transpose(aT_ps, a_sb[:, ic], ident)
                nc.scalar.activation(aT_sb[:, ic], aT_ps, AF.Copy)

            aT_flat = aT_sb.rearrange("q a b -> q (a b)")
            for fc in range(KH):
                c2q_ps = aps.tile([P, T], F32, tag="aps")
                nc.tensor.matmul(c2q_ps, lhsT=q_sb[:, ts(fc, P)], rhs=aT_flat,
                                 start=True, stop=True)
                c2q_sb = att.tile([P, T], F32, tag="c2q_sb")
                nc.scalar.activation(c2q_sb, c2q_ps, AF.Copy)
                c2qb_sb = att.tile([P, T], BF16, tag="c2qb_sb")
                nc.scalar.activation(c2qb_sb, c2q_ps, AF.Copy)
                g3_sb = att.tile([P, T], BF16, tag="g3")
                nc.vector.tensor_tensor(g3_sb, cT_sb[:, fc], c2q_sb, ALU.mult)
                g4_sb = att.tile([P, T], BF16, tag="g4")
                nc.vector.tensor_tensor(
                    g4_sb, cT_sb[:, fc],
                    q2cT_sb[:, fc, None].to_broadcast((P, T)), ALU.mult)
                nc.sync.dma_start(gT_d[b, fc], io["cT_bf"][b, fc])
                nc.sync.dma_start(gT_d[b, KH + fc], c2qb_sb)
                nc.sync.dma_start(gT_d[b, 2 * KH + fc], g3_sb)
                nc.sync.dma_start(gT_d[b, 3 * KH + fc], g4_sb)

    # ================ Phase 2: three layers ================
    for li, lname in enumerate(LAYERS):
        KC = KH4 if li == 0 else KH2

        # ---- 2a: xg = src @ wihT(x64, col-permuted) -> xg_d[lname] ----
        with tc.tile_pool(name=f"prj{li}", bufs=2) as prj, \
             tc.tile_pool(name=f"prjw{li}", bufs=1) as prjw, \
             tc.tile_pool(name=f"prj{li}_ps", bufs=2, space="PSUM") as pps:
            halves = 2 if li == 0 else 1
            HN = H4 // halves
            NB = HN // 512
            for half in range(halves):
                w_sb = prjw.tile([P, KC, HN], BF16, tag="wih")
                nc.sync.dma_start(
                    w_sb,
                    io[f"{lname}_wihT"][:, :, ds(half * HN, HN)]
                    .rearrange("kc p n -> p kc n"))
                for b in range(B):
                    for mc in range(T // P):
                        inp_sb = prj.tile([P, KC, P], BF16, tag="inp")
                        if li == 0:
                            nc.sync.dma_start(
                                inp_sb,
                                gT_d[b, :, :, ts(mc, P)].rearrange("kc p t -> p kc t"))
                        else:
                            prev = LAYERS[li - 1]
                            nc.sync.dma_start(inp_sb[:, :KH], mT_d[prev][:, :, b, ts(mc, P)])
                            nc.gpsimd.dma_start(inp_sb[:, KH:], rsout_d[prev][:, :, b, ts(mc, P)])
                        for n in range(NB):
                            xg_ps = pps.tile([P, 512], F32, tag="xg")
                            for k in range(KC):
                                nc.tensor.matmul(
                                    xg_ps, lhsT=inp_sb[:, k],
                                    rhs=w_sb[:, k, ts(n, 512)],
                                    start=(k == 0), stop=(k == KC - 1))
                            xg_sb = prj.tile([P, 512], FP8, tag="xg_sb")
                            nc.scalar.activation(xg_sb, xg_ps, AF.Copy)
                            off = half * HN + n * 512
                            nc.sync.dma_start(
                                xg_d[lname][ts(mc, P), b, ds(off, 512)],
                                xg_sb)

        # ---- 2b: scan (always "forward" in local time) ----
        with tc.tile_pool(name=f"whh{li}", bufs=1) as whhp, \
             tc.tile_pool(name=f"st{li}", bufs=1) as stp, \
             tc.tile_pool(name=f"scan{li}", bufs=2) as scp, \
             tc.tile_pool(name=f"xg{li}", bufs=SC, space="SBUF") as xgp, \
             tc.tile_pool(name=f"scan{li}_ps", bufs=1, space="PSUM") as sps, \
             tc.tile_pool(name=f"tp{li}_ps", bufs=1, space="PSUM") as tps:
            whh_sb = whhp.tile([P, KPAIR, 2, H4], FP8, name="whh_sb")
            nc.sync.dma_start(whh_sb, io[f"{lname}_whh8"].rearrange("a b p n -> p a b n"))

            hT8 = stp.tile([P, KPAIR, 2, PB], FP8, name="hT8")
            nc.vector.memset(hT8, 0.0)
            c_st = stp.tile([B, H], BF16, name="c_st")
            nc.vector.memset(c_st, 0.0)
            xring = []
            for j in range(8):
                xt = stp.tile([P, 2, H4], FP8, name=f"xring{j}")
                nc.vector.memset(xt, 0.0)
                xring.append(xt)

            with tc.For_i(0, NIT, 1) as iv:
                def issue_xg(j0):
                    for j in range(j0, min(j0 + 8, SC)):
                        (nc.sync if j % 2 == 0 else nc.gpsimd).dma_start(
                            xring[j % 8][:B, 0],
                            xg_d[lname][ds(iv * SC + j, 1)].rearrange("a b n -> (a b) n"))
                issue_xg(0)

                hacc = scp.tile([P, KH, B, SC], BF16, tag="hacc", name="hacc")
                hrev = scp.tile([P, KH, B, SC], BF16, tag="hrev", name="hrev")

                prev = None  # (hTa, hTb) transposes of previous step pending pack
                for j in range(SC):
                    gA = sps.tile([PB, 3, 512], F32, tag="gA", name="gA")
                    gB = sps.tile([PB, 3, 512], F32, tag="gB", name="gB")
                    xt = xring[j % 8]
                    # fold xg into PSUM via one-hot lhsT (independent of h -> fills tail stall)
                    for nb in range(3):
                        nc.tensor.matmul(gA[:, nb], lhsT=onehot_sb, rhs=xt[:, :, ts(nb, 512)],
                                         start=True, stop=False, perf_mode=PM.DoubleRow)
                    for nb in range(3):
                        nc.tensor.matmul(gB[:, nb], lhsT=onehot_sb, rhs=xt[:, :, ts(3 + nb, 512)],
                                         start=True, stop=False, perf_mode=PM.DoubleRow)
                    # previous step's transposes + packs (emitted here so this step's
                    # folds precede them on PE; kp mms below wait on these packs)
                    if prev is not None:
                        _emit_pack(nc, tc, prev, hT8, hacc, hrev, j - 1)
                        prev = None
                    # recurrent accumulation
                    for kp in range(KPAIR):
                        for g, nbl in ((gA, 0), (gB, 3)):
                            for nb in range(3):
                                nc.tensor.matmul(g[:, nb], lhsT=hT8[:, kp],
                                                 rhs=whh_sb[:, kp, :, ts(nbl + nb, 512)],
                                                 start=False, stop=(kp == KPAIR - 1),
                                                 perf_mode=PM.DoubleRow)
                    if j + 8 < SC:
                        ((nc.sync if j % 2 == 0 else nc.gpsimd)).dma_start(
                            xring[j % 8][:B, 0],
                            xg_d[lname][ds(iv * SC + j + 8, 1)].rearrange("a b n -> (a b) n"))
                    gAf = gA[:B].rearrange("p a n -> p (a n)")
                    gBf = gB[:B].rearrange("p a n -> p (a n)")
                    of_bf = scp.tile([B, 2, H], BF16, tag="of", name="of_bf")
                    nc.scalar.activation(of_bf[:, 1], gAf[:, H:], AF.Sigmoid, scale=ISC)
                    i_bf = scp.tile([B, H], BF16, tag="i_bf", name="i_bf")
                    nc.scalar.activation(i_bf, gBf[:, :H], AF.Sigmoid, scale=ISC)
                    # tail: Act order sf,si,g0,g1,so,tc0,tc1 ; DVE cf/ig/ca/h per sub
                    gs_t, ig_t, tc_t, hs_t = [], [], [], []
                    for lo_c, wd, si in ((0, 512, 0), (512, 256, 1)):
                        cs = c_st[:, ds(lo_c, wd)]
                        nc.vector.tensor_tensor(cs, of_bf[:, 1, ds(lo_c, wd)], cs, ALU.mult)
                        gs = scp.tile([B, wd], BF16, tag=f"gs{si}", name=f"gs{si}")
                        nc.scalar.activation(gs, gBf[:, ds(H + lo_c, wd)], AF.Tanh, scale=ISC)
                        gs_t.append(gs)
                    for lo_c, wd, si in ((0, 512, 0), (512, 256, 1)):
                        cs = c_st[:, ds(lo_c, wd)]
                        ig = scp.tile([B, wd], BF16, tag=f"ig{si}", name=f"ig{si}")
                        nc.vector.tensor_tensor(ig, i_bf[:, ds(lo_c, wd)], gs_t[si], ALU.mult)
                        nc.vector.tensor_tensor(cs, cs, ig, ALU.add)
                    nc.scalar.activation(of_bf[:, 0], gAf[:, :H], AF.Sigmoid, scale=ISC)
                    subs = []
                    for lo_c, wd, si in ((0, 512, 0), (512, 256, 1)):
                        cs = c_st[:, ds(lo_c, wd)]
                        tcs = scp.tile([B, wd], BF16, tag=f"tc{si}", name=f"tc{si}")
                        nc.scalar.activation(tcs, cs, AF.Tanh)
                        hs = scp.tile([B, wd], BF16, tag=f"hs{si}", name=f"hs{si}")
                        nc.vector.tensor_tensor(hs, of_bf[:, 0, ds(lo_c, wd)], tcs, ALU.mult)
                        subs.append(hs)
                    hTa = tps.tile([P, 4, B], BF16, tag="hTa", name="hTa")
                    hTb = tps.tile([P, 2, B], BF16, tag="hTb", name="hTb")
                    for k in range(4):
                        nc.tensor.transpose(hTa[:, k], subs[0][:, ts(k, P)], ident_bf[:B, :B])
                    for k in (0, 1):
                        nc.tensor.transpose(hTb[:, k], subs[1][:, ts(k, P)], ident_bf[:B, :B])
                    prev = (hTa, hTb)
                _emit_pack(nc, tc, prev, hT8, hacc, hrev, SC - 1)

                # flush: own order -> mT_d ; reversed+masked -> rsin_d shards
                nc.scalar.dma_start(mT_d[lname][:, :, :, ds(iv * SC, SC)], hacc)
                hs0 = scp.tile([P, KH, B, SC], BF16, tag="hs0", name="hs0")
                nc.vector.tensor_scalar_mul(hs0, hrev, mask_sb[:, 0, None])
                hs1 = scp.tile([P, KH, B, SC], BF16, tag="hs1", name="hs1")
                nc.vector.tensor_scalar_mul(hs1, hrev, mask_sb[:, 1, None])
                nc.scalar.dma_start(rsin_d[lname][0][:, :, :, ds(T - SC - iv * SC, SC)], hs0)
                nc.scalar.dma_start(rsin_d[lname][1][:, :, :, ds(T - SC - iv * SC, SC)], hs1)

        # ---- 2c: pair exchange ----
        nc.gpsimd.collective_compute(
            "ReduceScatter", mybir.AluOpType.add,
            replica_groups=[[0, 1], [2, 3], [4, 5], [6, 7]],
            ins=[rsin_d[lname].rearrange("s p k b t -> (s p) (k b t)").opt()],
            outs=[rsout_d[lname].rearrange("p k b t -> p (k b t)").opt()],
        )

    # ================ Phase 3: p readout ================
    with tc.tile_pool(name="out", bufs=3) as osb, \
         tc.tile_pool(name="out_ps", bufs=2, space="PSUM") as ops:
        for b in range(B):
            p_ps = ops.tile([1, T], F32, tag="p_ps", name="p_ps")
            for k in range(KH4):
                gt = osb.tile([P, T], BF16, tag="gt")
                nc.sync.dma_start(gt, gT_d[b, k])
                nc.tensor.matmul(p_ps, lhsT=pw_sb["pwg"][:, k, None],
                                 rhs=gt, start=(k == 0), stop=False)
            for nm, own, peer in (("pwm2", mT_d["l2"], rsout_d["l2"]),
                                  ("pwmo", mT_d["lo"], rsout_d["lo"])):
                for k in range(KH2):
                    mt = osb.tile([P, T], BF16, tag=f"mt_{nm}")
                    src = own if k < KH else peer
                    nc.sync.dma_start(mt, src[:, k % KH, b])
                    nc.tensor.matmul(p_ps, lhsT=pw_sb[nm][:, k, None],
                                     rhs=mt, start=False,
                                     stop=(nm == "pwmo" and k == KH2 - 1))
            p_sb = osb.tile([1, T], F32, tag="p_sb")
            nc.scalar.activation(p_sb, p_ps, AF.Copy, bias=float(p_bias))
            nc.sync.dma_start(io["p"][b], p_sb)


# ==================== host-side driver ====================

_GATE_PERM = None


def _gate_perm():
    """column permutation: new [o f i g~] from torch (i,f,g,o)."""
    global _GATE_PERM
    if _GATE_PERM is None:
        o = np.arange(3 * H, 4 * H)
        f = np.arange(H, 2 * H)
        i = np.arange(0, H)
        g = np.arange(2 * H, 3 * H)
        _GATE_PERM = np.concatenate([o, f, i, g])
    return _GATE_PERM


def _prep_core(inputs, core):
    import ml_dtypes
    bf16 = ml_dtypes.bfloat16
    f8 = ml_dtypes.float8_e4m3
    f32 = np.float32
    pair, is_b = core // 2, core % 2
    lo, hi = pair * B, (pair + 1) * B
    d = "b" if is_b else "f"
    perm = _gate_perm()

    m = {}
    cs = np.asarray(inputs["c"][lo:hi], f32)
    if is_b:
        cs = cs[:, ::-1]
    qs = np.asarray(inputs["q"][lo:hi], f32)
    cT = np.ascontiguousarray(cs.transpose(0, 2, 1).reshape(B, KH, P, T))
    m["c"] = np.ascontiguousarray(cs)
    m["q"] = np.ascontiguousarray(qs)
    m["cT"] = cT
    m["cT_bf"] = cT.astype(bf16)
    m["qT"] = np.ascontiguousarray(qs.transpose(0, 2, 1).reshape(B, KH, P, QLEN))

    m["w_cq_"] = np.ascontiguousarray(inputs["w_att_cq"].reshape(KH, P).T).astype(f32)
    m["w_c_"] = np.ascontiguousarray(inputs["w_att_c"].reshape(KH, P).T).astype(f32)
    m["w_q_"] = np.ascontiguousarray(inputs["w_att_q"].reshape(KH, P).T).astype(f32)

    for lname in LAYERS:
        wih = np.asarray(inputs[f"{lname}{d}_wih"], f32)   # [4H, in]
        whh = np.asarray(inputs[f"{lname}{d}_whh"], f32)   # [4H, H]
        ind = wih.shape[1]
        wihT = wih.T[:, perm] * SCALE                      # [in, 4H] x64, col-perm
        if lname != "l1":
            # rows: own-dir half first, peer half second
            top, bot = wihT[:H], wihT[H:]
            wihT = np.concatenate([bot, top], 0) if is_b else wihT
        m[f"{lname}_wihT"] = np.ascontiguousarray(
            wihT.reshape(ind // P, P, H4)).astype(bf16)
        whhT = whh.T[:, perm] * SCALE                      # [H, 4H]
        m[f"{lname}_whh8"] = np.ascontiguousarray(
            whhT.reshape(KPAIR, 2, P, H4)).astype(f8)

    if is_b:
        wg, wm = np.asarray(inputs["p2_wg"], f32), np.asarray(inputs["p2_wm"], f32)
        wm_loc = np.concatenate([wm[H:], wm[:H]])
        wm2, wmo = np.zeros(H2, f32), wm_loc
    else:
        wg, wm = np.asarray(inputs["p1_wg"], f32), np.asarray(inputs["p1_wm"], f32)
        wm2, wmo = wm, np.zeros(H2, f32)
    m["pwg"] = np.ascontiguousarray(wg.reshape(KH4, P).T).astype(bf16)
    m["pwm2"] = np.ascontiguousarray(wm2.reshape(KH2, P).T).astype(bf16)
    m["pwmo"] = np.ascontiguousarray(wmo.reshape(KH2, P).T).astype(bf16)

    mk = np.zeros((P, 2), f32)
    mk[:, 1 - is_b] = 1.0   # even core contributes shard1; odd shard0
    m["mask"] = mk
    oh = np.zeros((P, 2, PB), np.float32)
    for k in range(B):
        oh[k, 0, k] = 1.0
    m["onehot"] = oh.astype(f8)
    return m


def declare_io(nc):
    io = {}

    def inp(name, shape, dt=F32):
        io[name] = nc.declare_dram_parameter(name, list(shape), dt, isOutput=False).ap()

    inp("c", (B, T, H))
    inp("q", (B, QLEN, H))
    inp("cT", (B, KH, P, T))
    inp("cT_bf", (B, KH, P, T), BF16)
    inp("qT", (B, KH, P, QLEN))
    inp("w_cq_", (P, KH))
    inp("w_c_", (P, KH))
    inp("w_q_", (P, KH))
    inp("mask", (P, 2))
    inp("onehot", (P, 2, PB), FP8)
    for lname in LAYERS:
        ind = H4 if lname == "l1" else H2
        inp(f"{lname}_wihT", (ind // P, P, H4), BF16)
        inp(f"{lname}_whh8", (KPAIR, 2, P, H4), FP8)
    inp("pwg", (P, KH4), BF16)
    inp("pwm2", (P, KH2), BF16)
    inp("pwmo", (P, KH2), BF16)
    io["p"] = nc.declare_dram_parameter("p", [B, T], F32, isOutput=True).ap()
    return io


def kernel(**inputs):
    from concourse.bass_utils import run_bass_kernel_spmd

    cfg = {
        "b_att": float(inputs["b_att_c"]) + float(inputs["b_att_q"]) + float(inputs["b_att_cq"]),
        "p_bias": 0.0,  # per-core below
    }
    p1_b = float(inputs["p1_bg"]) + float(inputs["p1_bm"])
    p2_b = float(inputs["p2_bg"]) + float(inputs["p2_bm"])

    nc = bacc.Bacc("TRN2", target_bir_lowering=False, debug=False, num_devices=N_CORES)
    io = declare_io(nc)
    cfg["p_bias"] = 0.0
    with tile.TileContext(nc) as tc, ExitStack() as ctx:
        build(ctx, tc, io, cfg)
    nc.compile()

    in_maps = [_prep_core(inputs, core) for core in range(N_CORES)]

    import os as _os
    import time as _time

    def _run():
        try:
            return run_bass_kernel_spmd(nc, in_maps, core_ids=list(range(N_CORES)))
        except Exception:
            if _os.environ.get("BASS_TRACE"):
                _os.environ["BASS_NEVER_TRACE"] = "1"
                return run_bass_kernel_spmd(nc, in_maps, core_ids=list(range(N_CORES)))
            raise

    t0 = _time.time()
    res = _run()
    globals()["LAST_RUN"] = res
    globals()["LAST_EXEC_WALL"] = _time.time() - t0
    t0 = _time.time()
    res2 = _run()
    globals()["WARM_EXEC_WALL"] = _time.time() - t0
    if res2.exec_time_ns is not None:
        globals()["LAST_RUN"] = res2
    res = res2

    p1 = np.concatenate([res.results[2 * g]["p"] + p1_b for g in range(4)], axis=0)
    p2 = np.concatenate([res.results[2 * g + 1]["p"][:, ::-1] + p2_b for g in range(4)], axis=0)
    return p1, p2
